# revision 7
# baseline (speedup 1.0000x reference)
# Triplet-margin loss kernel for Trainium2 (Bass/Tile), batch-sharded
# across 8 NeuronCores.
#
# reference math (torch F.pairwise_distance semantics):
#   d_ap[b,p] = || anc[b] - pos[b,p] + eps ||_2
#   d_an[b,n] = || anc[b] - neg[b,n] + eps ||_2
#   loss = mean_{b,p,n} max(d_ap[b,p] - d_an[b,n] + margin, 0)
#
# Per 128-row batch tile there are 24 distance columns ("slices"), each a
# [128, 1024] fp32 read. The kernel is HBM-DMA-bound (~25 MiB/core), so
# the slices are spread across three engine paths sized so every engine's
# busy time (~28-30 us/tile) stays under the per-tile DMA time (~35 us):
#   T1 (jj 0-14):  dot a'.x on DVE (stt fp32, accum) + ||x||^2 on ACT
#       (Square, accum); d^2 = nrm - 2 dot + ||a'||^2.
#   W  (jj 15-19): w = x - 2a' on GpSimd, then sum x*w on DVE
#       (= d^2 - ||a'||^2); the dot column is zeroed so the shared
#       combine yields d.
#   B  (jj 20-23): u = x - a' on GpSimd (bf16 out), sum u^2 on ACT
#       (= d^2 directly; bias-0 sqrt).
# Trailing-read fix: a DVE op whose input tile is still being written by
# GpSimd starts early via subtile deps and stretches to the GpSimd op's
# duration (~3.3 us instead of ~1.2 us, measured). Each W stt therefore
# takes its unit scalar from a tiny [P,1] token tile that GpSimd memsets
# right after the subtract: the token gates the DVE op until the operand
# is fully written, keeping the stt at full rate. ACT waits for complete
# operands on its own, so B needs no token.
# The (p,n) pairing uses scalar_tensor_tensor(subtract, min 0, accum)
# on DVE, which yields -sum_n relu(d_ap - d_an + margin) per (b,p).
# Chunks are 4 slices = 2 MB with 16 KB/partition descriptors; the
# GpSimd-owned chunks are DMA'd first, and the last (pos) chunk is split
# in two to shorten the end-of-kernel tail.
# Each core returns per-partition partial sums [128, 2]; the host sums
# and scales.

import numpy as np

import concourse.bacc as bacc
import concourse.mybir as mybir
import concourse.tile as tile
from concourse import bass_utils

B, Z = 2048, 1024
NUM_POS, NUM_NEG = 8, 16
NJ = NUM_POS + NUM_NEG
MARGIN, EPS = 1.0, 1e-6
N_CORES = 8
BL = B // N_CORES  # 256 rows of anc per core
P = 128
NT = BL // P  # 2 batch-tiles per core
CH = 4  # z-slices per full DMA chunk

# slice-type split by jj: [0, W_START) T1, [W_START, B_START) W, rest B
W_START = 15
B_START = 20
XP_BUFS = 8

F32 = mybir.dt.float32
BF16 = mybir.dt.bfloat16
AF = mybir.ActivationFunctionType
OP = mybir.AluOpType

# chunk list: (first_jj, n_slices), DMA-issued in this order. GpSimd-owned
# slices (jj >= 15) first; the last chunk is split to shorten the tail.
CHUNKS = [(20, 4), (16, 4), (12, 4), (8, 4), (4, 4), (0, 2), (2, 2)]


def _emit(tc, nc, anc, pos, neg, out):
    v = nc.vector
    act = nc.scalar
    gp = nc.gpsimd
    pos2 = pos.rearrange("(b j) z -> b (j z)", j=NUM_POS)  # [BL, 8*Z]
    neg2 = neg.rearrange("(b j) z -> b (j z)", j=NUM_NEG)  # [BL, 16*Z]
    with (
        tc.tile_pool(name="xp", bufs=XP_BUFS) as xp,
        tc.tile_pool(name="up", bufs=4) as up,
        tc.tile_pool(name="tkp", bufs=4) as tkp,
        tc.tile_pool(name="apool", bufs=2) as apool,
        tc.tile_pool(name="scp", bufs=1) as scp,
        tc.tile_pool(name="smp", bufs=2) as smp,
        tc.tile_pool(name="opool", bufs=1) as opool,
    ):
        osb = opool.tile([P, NT], F32, name="osb")
        dve_scr = scp.tile([P, Z], F32, name="dve_scr")
        act_scr = scp.tile([P, Z], BF16, name="act_scr")
        ts_out = scp.tile([P, NUM_NEG], F32, name="ts_out")
        zero_n = opool.tile([P, NUM_NEG], F32, name="zero_n")
        v.memset(zero_n[:, :], 0.0)

        # prologue: both tiles' anc loads, a' = anc + eps, a2 = 2a', ||a'||^2
        aprimes, a2s, anrms = [], [], []
        for t in range(NT):
            b0 = t * P
            anc_in = apool.tile([P, Z], F32, name="anc_in")
            aprime = apool.tile([P, Z], F32, name="aprime")
            a2 = apool.tile([P, Z], F32, name="a2")
            a_nrm = smp.tile([P, 1], F32, name="a_nrm")
            nc.sync.dma_start(anc_in[:, :], anc[b0 : b0 + P, :])
            v.tensor_scalar_add(aprime[:, :], anc_in[:, :], EPS)
            v.tensor_scalar_mul(a2[:, :], aprime[:, :], 2.0)
            act.activation(
                act_scr[:, :], aprime[:, :], AF.Square, accum_out=a_nrm[:, 0:1]
            )
            aprimes.append(aprime)
            a2s.append(a2)
            anrms.append(a_nrm)

        for t in range(NT):
            b0 = t * P
            aprime = aprimes[t]
            a2 = a2s[t]
            a_nrm = anrms[t]
            dot = smp.tile([P, B_START], F32, name="dot")
            nrm = smp.tile([P, NJ], F32, name="nrm")
            d2c = smp.tile([P, B_START], F32, name="d2c")
            dt_ = smp.tile([P, NJ], F32, name="dt_")
            s_m = smp.tile([P, NUM_POS], F32, name="s_m")
            lp = smp.tile([P, NUM_POS], F32, name="lp")
            # W cols contribute nothing via dot in the shared combine
            v.memset(dot[:, W_START:B_START], 0.0)

            tiles = []
            for jj0, nsl in CHUNKS:
                w = nsl * Z
                xt = xp.tile([P, CH * Z], F32, name="xt")
                if jj0 < NUM_POS:
                    src = pos2[b0 : b0 + P, jj0 * Z : jj0 * Z + w]
                else:
                    src = neg2[
                        b0 : b0 + P, (jj0 - NUM_POS) * Z : (jj0 - NUM_POS) * Z + w
                    ]
                nc.sync.dma_start(xt[:, 0:w], src)
                tiles.append(xt)

            for (jj0, nsl), xt in zip(CHUNKS, tiles):
                for q in range(nsl):
                    jj = jj0 + q
                    xs = xt[:, q * Z : (q + 1) * Z]
                    if jj < W_START:
                        v.scalar_tensor_tensor(
                            out=dve_scr[:, :],
                            in0=xs,
                            scalar=1.0,
                            in1=aprime[:, :],
                            op0=OP.bypass,
                            op1=OP.mult,
                            accum_out=dot[:, jj : jj + 1],
                        )
                        act.activation(
                            act_scr[:, :], xs, AF.Square, accum_out=nrm[:, jj : jj + 1]
                        )
                    elif jj < B_START:
                        wt = up.tile([P, Z], F32, name="wt")
                        tok = tkp.tile([P, 1], F32, name="tok")
                        gp.tensor_tensor(
                            out=wt[:, :], in0=xs, in1=a2[:, :], op=OP.subtract
                        )
                        gp.memset(tok[:, :], 1.0)
                        # sum x*(x - 2a') = d^2 - ||a'||^2; the tok scalar
                        # gates the stt until wt is fully written
                        v.scalar_tensor_tensor(
                            out=dve_scr[:, :],
                            in0=xs,
                            scalar=tok[:, 0:1],
                            in1=wt[:, :],
                            op0=OP.mult,
                            op1=OP.mult,
                            accum_out=nrm[:, jj : jj + 1],
                        )
                    else:
                        ut = up.tile([P, Z], BF16, name="ut")
                        gp.tensor_tensor(
                            out=ut[:, :], in0=xs, in1=aprime[:, :], op=OP.subtract
                        )
                        act.activation(
                            act_scr[:, :],
                            ut[:, :],
                            AF.Square,
                            accum_out=nrm[:, jj : jj + 1],
                        )

            # T1/W cols: d = sqrt((nrm - 2*dot) + ||a'||^2)
            v.scalar_tensor_tensor(
                out=d2c[:, :],
                in0=dot[:, :],
                scalar=-2.0,
                in1=nrm[:, 0:B_START],
                op0=OP.mult,
                op1=OP.add,
            )
            act.activation(
                dt_[:, 0:B_START],
                d2c[:, :],
                AF.Sqrt,
                bias=a_nrm[:, 0:1],
                scale=1.0,
            )
            # B cols already hold d^2 in nrm
            act.activation(dt_[:, B_START:NJ], nrm[:, B_START:NJ], AF.Sqrt)
            # s = d_ap + margin
            v.tensor_scalar_add(s_m[:, :], dt_[:, 0:NUM_POS], MARGIN)
            # lp[:,p] = sum_n min(d_an - s_p, 0) = -sum_n relu(s_p - d_an)
            for p_i in range(NUM_POS):
                v.scalar_tensor_tensor(
                    out=ts_out[:, :],
                    in0=dt_[:, NUM_POS:NJ],
                    scalar=s_m[:, p_i : p_i + 1],
                    in1=zero_n[:, :],
                    op0=OP.subtract,
                    op1=OP.min,
                    accum_out=lp[:, p_i : p_i + 1],
                )
            v.reduce_sum(osb[:, t : t + 1], lp[:, :], axis=mybir.AxisListType.X)
        nc.sync.dma_start(out[:, :], osb[:, :])


_NC_CACHE = None


def build():
    global _NC_CACHE
    if _NC_CACHE is None:
        nc = bacc.Bacc(
            "TRN2", target_bir_lowering=False, debug=False, num_devices=N_CORES
        )
        anc = nc.dram_tensor("anc", (BL, Z), F32, kind="ExternalInput").ap()
        pos = nc.dram_tensor("pos", (BL * NUM_POS, Z), F32, kind="ExternalInput").ap()
        neg = nc.dram_tensor("neg", (BL * NUM_NEG, Z), F32, kind="ExternalInput").ap()
        out = nc.dram_tensor("out", (P, NT), F32, kind="ExternalOutput").ap()
        with tile.TileContext(nc) as tc:
            _emit(tc, nc, anc, pos, neg, out)
        nc.compile()
        _NC_CACHE = nc
    return _NC_CACHE


def make_in_maps(anc_embedding, pos_embedding, neg_embedding):
    anc_embedding = np.asarray(anc_embedding, dtype=np.float32)
    pos_embedding = np.asarray(pos_embedding, dtype=np.float32)
    neg_embedding = np.asarray(neg_embedding, dtype=np.float32)
    in_maps = []
    for c in range(N_CORES):
        in_maps.append(
            {
                "anc": np.ascontiguousarray(anc_embedding[c * BL : (c + 1) * BL]),
                "pos": np.ascontiguousarray(
                    pos_embedding[c * BL * NUM_POS : (c + 1) * BL * NUM_POS]
                ),
                "neg": np.ascontiguousarray(
                    neg_embedding[c * BL * NUM_NEG : (c + 1) * BL * NUM_NEG]
                ),
            }
        )
    return in_maps


def combine(outs):
    # outs: list of [P, NT] per-core partial sums of min(d_an - s, 0)
    total = sum(o.astype(np.float64).sum() for o in outs)
    return np.float32(-total / (B * NUM_POS * NUM_NEG))


def kernel(anc_embedding, pos_embedding, neg_embedding):
    nc = build()
    in_maps = make_in_maps(anc_embedding, pos_embedding, neg_embedding)
    res = bass_utils.run_bass_kernel_spmd(nc, in_maps, core_ids=list(range(N_CORES)))
    return combine([r["out"] for r in res.results])


# revision 8
# speedup vs baseline: 1.2927x; 1.2927x over previous
# Triplet-margin loss kernel for Trainium2 (Bass/Tile), batch-sharded
# across 8 NeuronCores.
#
# reference math (torch F.pairwise_distance semantics):
#   d_ap[b,p] = || anc[b] - pos[b,p] + eps ||_2
#   d_an[b,n] = || anc[b] - neg[b,n] + eps ||_2
#   loss = mean_{b,p,n} max(d_ap[b,p] - d_an[b,n] + margin, 0)
#
# Per 128-row batch tile there are 24 distance columns ("slices"), each a
# [128, 1024] fp32 read. Every slice uses the same two-engine path:
#   dot a'.x on DVE (scalar_tensor_tensor fp32, accum_out) and ||x||^2 on
#   ACT (activation Square, accum_out); d = sqrt(nrm - 2 dot + ||a'||^2).
# With that split DVE (~33 us/tile), ACT (~34 us/tile) and DMA
# (~35 us/tile) are all at the roofline ridge.
# GpSimd is deliberately UNUSED: measurements show DVE ops stretch from
# ~1.2 us to ~3.2 us while GpSimd runs (shared SBUF ports), so any work
# routed through GpSimd lowers total elementwise throughput.
# The (p,n) pairing uses scalar_tensor_tensor(subtract, min 0, accum)
# on DVE, which yields -sum_n relu(d_ap - d_an + margin) per (b,p).
# Chunks are 4 slices = 2 MB with 16 KB/partition descriptors; the last
# chunk is split in two to shorten the end-of-kernel tail.
# Each core returns per-partition partial sums [128, 2]; the host sums
# and scales.

import numpy as np

import concourse.bacc as bacc
import concourse.mybir as mybir
import concourse.tile as tile
from concourse import bass_utils

B, Z = 2048, 1024
NUM_POS, NUM_NEG = 8, 16
NJ = NUM_POS + NUM_NEG
MARGIN, EPS = 1.0, 1e-6
N_CORES = 8
BL = B // N_CORES  # 256 rows of anc per core
P = 128
NT = BL // P  # 2 batch-tiles per core
CH = 4  # z-slices per full DMA chunk
XP_BUFS = 10

F32 = mybir.dt.float32
BF16 = mybir.dt.bfloat16
AF = mybir.ActivationFunctionType
OP = mybir.AluOpType

# chunk list: (first_jj, n_slices), DMA-issued in this order; the last
# chunk is split to shorten the tail.
CHUNKS = [(0, 4), (4, 4), (8, 4), (12, 4), (16, 4), (20, 2), (22, 2)]


def _emit(tc, nc, anc, pos, neg, out):
    v = nc.vector
    act = nc.scalar
    pos2 = pos.rearrange("(b j) z -> b (j z)", j=NUM_POS)  # [BL, 8*Z]
    neg2 = neg.rearrange("(b j) z -> b (j z)", j=NUM_NEG)  # [BL, 16*Z]
    with (
        tc.tile_pool(name="xp", bufs=XP_BUFS) as xp,
        tc.tile_pool(name="apool", bufs=2) as apool,
        tc.tile_pool(name="scp", bufs=1) as scp,
        tc.tile_pool(name="smp", bufs=2) as smp,
        tc.tile_pool(name="opool", bufs=1) as opool,
    ):
        osb = opool.tile([P, NT], F32, name="osb")
        dve_scr = scp.tile([P, Z], F32, name="dve_scr")
        act_scr = scp.tile([P, Z], BF16, name="act_scr")
        ts_out = scp.tile([P, NUM_NEG], F32, name="ts_out")
        zero_n = opool.tile([P, NUM_NEG], F32, name="zero_n")
        v.memset(zero_n[:, :], 0.0)

        # prologue: both tiles' anc loads, a' = anc + eps, ||a'||^2
        aprimes, anrms = [], []
        for t in range(NT):
            b0 = t * P
            anc_in = apool.tile([P, Z], F32, name="anc_in")
            aprime = apool.tile([P, Z], F32, name="aprime")
            a_nrm = smp.tile([P, 1], F32, name="a_nrm")
            nc.sync.dma_start(anc_in[:, :], anc[b0 : b0 + P, :])
            v.tensor_scalar_add(aprime[:, :], anc_in[:, :], EPS)
            act.activation(
                act_scr[:, :], aprime[:, :], AF.Square, accum_out=a_nrm[:, 0:1]
            )
            aprimes.append(aprime)
            anrms.append(a_nrm)

        for t in range(NT):
            b0 = t * P
            aprime = aprimes[t]
            a_nrm = anrms[t]
            dot = smp.tile([P, NJ], F32, name="dot")
            nrm = smp.tile([P, NJ], F32, name="nrm")
            d2c = smp.tile([P, NJ], F32, name="d2c")
            dt_ = smp.tile([P, NJ], F32, name="dt_")
            s_m = smp.tile([P, NUM_POS], F32, name="s_m")
            lp = smp.tile([P, NUM_POS], F32, name="lp")

            tiles = []
            for jj0, nsl in CHUNKS:
                w = nsl * Z
                xt = xp.tile([P, CH * Z], F32, name="xt")
                if jj0 < NUM_POS:
                    src = pos2[b0 : b0 + P, jj0 * Z : jj0 * Z + w]
                else:
                    src = neg2[
                        b0 : b0 + P, (jj0 - NUM_POS) * Z : (jj0 - NUM_POS) * Z + w
                    ]
                nc.sync.dma_start(xt[:, 0:w], src)
                tiles.append(xt)

            for (jj0, nsl), xt in zip(CHUNKS, tiles):
                for q in range(nsl):
                    jj = jj0 + q
                    xs = xt[:, q * Z : (q + 1) * Z]
                    v.scalar_tensor_tensor(
                        out=dve_scr[:, :],
                        in0=xs,
                        scalar=1.0,
                        in1=aprime[:, :],
                        op0=OP.bypass,
                        op1=OP.mult,
                        accum_out=dot[:, jj : jj + 1],
                    )
                    act.activation(
                        act_scr[:, :], xs, AF.Square, accum_out=nrm[:, jj : jj + 1]
                    )

            # d2c = nrm - 2*dot ; d = sqrt(d2c + ||a'||^2)
            v.scalar_tensor_tensor(
                out=d2c[:, :],
                in0=dot[:, :],
                scalar=-2.0,
                in1=nrm[:, :],
                op0=OP.mult,
                op1=OP.add,
            )
            act.activation(
                dt_[:, :], d2c[:, :], AF.Sqrt, bias=a_nrm[:, 0:1], scale=1.0
            )
            # s = d_ap + margin
            v.tensor_scalar_add(s_m[:, :], dt_[:, 0:NUM_POS], MARGIN)
            # lp[:,p] = sum_n min(d_an - s_p, 0) = -sum_n relu(s_p - d_an)
            for p_i in range(NUM_POS):
                v.scalar_tensor_tensor(
                    out=ts_out[:, :],
                    in0=dt_[:, NUM_POS:NJ],
                    scalar=s_m[:, p_i : p_i + 1],
                    in1=zero_n[:, :],
                    op0=OP.subtract,
                    op1=OP.min,
                    accum_out=lp[:, p_i : p_i + 1],
                )
            v.reduce_sum(osb[:, t : t + 1], lp[:, :], axis=mybir.AxisListType.X)
        nc.sync.dma_start(out[:, :], osb[:, :])


_NC_CACHE = None


def build():
    global _NC_CACHE
    if _NC_CACHE is None:
        nc = bacc.Bacc(
            "TRN2", target_bir_lowering=False, debug=False, num_devices=N_CORES
        )
        anc = nc.dram_tensor("anc", (BL, Z), F32, kind="ExternalInput").ap()
        pos = nc.dram_tensor("pos", (BL * NUM_POS, Z), F32, kind="ExternalInput").ap()
        neg = nc.dram_tensor("neg", (BL * NUM_NEG, Z), F32, kind="ExternalInput").ap()
        out = nc.dram_tensor("out", (P, NT), F32, kind="ExternalOutput").ap()
        with tile.TileContext(nc) as tc:
            _emit(tc, nc, anc, pos, neg, out)
        nc.compile()
        _NC_CACHE = nc
    return _NC_CACHE


def make_in_maps(anc_embedding, pos_embedding, neg_embedding):
    anc_embedding = np.asarray(anc_embedding, dtype=np.float32)
    pos_embedding = np.asarray(pos_embedding, dtype=np.float32)
    neg_embedding = np.asarray(neg_embedding, dtype=np.float32)
    in_maps = []
    for c in range(N_CORES):
        in_maps.append(
            {
                "anc": np.ascontiguousarray(anc_embedding[c * BL : (c + 1) * BL]),
                "pos": np.ascontiguousarray(
                    pos_embedding[c * BL * NUM_POS : (c + 1) * BL * NUM_POS]
                ),
                "neg": np.ascontiguousarray(
                    neg_embedding[c * BL * NUM_NEG : (c + 1) * BL * NUM_NEG]
                ),
            }
        )
    return in_maps


def combine(outs):
    # outs: list of [P, NT] per-core partial sums of min(d_an - s, 0)
    total = sum(o.astype(np.float64).sum() for o in outs)
    return np.float32(-total / (B * NUM_POS * NUM_NEG))


def kernel(anc_embedding, pos_embedding, neg_embedding):
    nc = build()
    in_maps = make_in_maps(anc_embedding, pos_embedding, neg_embedding)
    res = bass_utils.run_bass_kernel_spmd(nc, in_maps, core_ids=list(range(N_CORES)))
    return combine([r["out"] for r in res.results])
